# revision 55
# baseline (speedup 1.0000x reference)
"""Trainium2 Bass kernel for nn_Encoder_52312701666158 (dense-GCN encoder).

Math (per graph):
    x   = concat(type_emb[types], label_emb[labels])          [N, 64]
    deg = clip(adj.sum(-1), 1, inf); dis = deg**-0.5
    H1  = relu(dis_i*(adj @ (dis_j*x)) @ W1 + b1)     (W1 deferred via associativity)
    H2  = relu(dis_i*(adj @ (dis_j*H1)) @ W2 + b2)
    out = concat(H2.mean(0), H2.max(0)) @ Wr.T + br           [64]

Sharding: data-parallel over the batch dim, 2 graphs per NeuronCore x 8 cores.

Device strategy (DMA-roofline oriented):
  * adj is shipped from the host pre-centered (adj - 0.5), pre-cast to
    fp8e4 (e4m3) and pre-TRANSPOSED.  The 0.5 offset is restored exactly
    via a rank-1 correction computed on-device (ones-matmul over Z), which
    halves the fp8 quantization error; with Z kept in fp16 the measured
    end-to-end L2 error of this scheme is ~3.5e-4 (gate is 2e-2).
  * The fp8 A.T [4096, 4096] = 16 MiB fits SBUF (128 KiB/partition), so it
    is loaded ONCE per graph (split across both HWDGE queues, ~400 GB/s)
    and stays RESIDENT for deg + BOTH GCN layers: adj traffic is
    1 x 16 MiB per graph instead of 3 x 64 MiB f32.
  * deg_i = sum_j A'.T[j,i] runs on the PE as a ones-matmul over the
    resident tiles, 4-way column-group tiled (4 concurrent rhs streams).
  * W1/W2 are applied AFTER the A-contraction ((A@Z)@W == A@(Z@W)), in
    d-major space [64, N], which kills the per-node-tile transpose+matmul
    chains; bias+relu+mean-pool all fold into the ACT PSUM drains.
  * The embedding lookup is ONE fused gather per node tile from a host-
    built [VOCAB*NTYPES, 64] product table (the two-table lookup would
    double the serialized SWDGE gather stream).
  * DMA queue discipline: sync+scalar HWDGE carry the big A.T streams
    and the dependent small dis-chain hops; gpsimd carries only the
    gathers and the output write, so no queue head-of-line-blocks another
    graph's work.
"""

import numpy as np
import ml_dtypes

import concourse.bass as bass
import concourse.bacc as bacc
import concourse.mybir as mybir
import concourse.tile as tile
from concourse import bass_utils
from concourse.masks import make_identity

B, N, D = 16, 4096, 64
NCORES = 8
BPC = B // NCORES          # graphs per core
NT = N // 128              # node tiles per graph
ND = NT // 2               # double-tiles (256 rows per DMA)
HALF = 2048                # i-chunk span per PSUM accumulator (4 banks)
VOCAB, NTYPES, EMB = 1000, 16, 32
GATHER_BATCH = False       # multi-column idx gathers fail the BIR verifier

F32 = mybir.dt.float32
FP16 = mybir.dt.float16
FP8 = mybir.dt.float8e4
I32 = mybir.dt.int32
AF = mybir.ActivationFunctionType

NP_FP8 = ml_dtypes.float8_e4m3

_CACHE = {}


def _build(BPC=BPC, N=N, NCORES=NCORES):
    NT = N // 128
    nc = bacc.Bacc("TRN2", target_bir_lowering=False, debug=False, num_devices=NCORES)

    a_t = nc.dram_tensor("a_t", [BPC, N, N], FP8, kind="ExternalInput").ap()
    fidx = nc.dram_tensor("fused_idx", [BPC, N], I32, kind="ExternalInput").ap()
    xtab = nc.dram_tensor("xtab", [VOCAB * NTYPES, D], FP16,
                          kind="ExternalInput").ap()
    w1 = nc.dram_tensor("W1h", [D, D], FP16, kind="ExternalInput").ap()
    w2 = nc.dram_tensor("W2h", [D, D], FP16, kind="ExternalInput").ap()
    b1 = nc.dram_tensor("b1", [D], F32, kind="ExternalInput").ap()
    b2 = nc.dram_tensor("b2", [D], F32, kind="ExternalInput").ap()
    wr = nc.dram_tensor("Wr", [D, 2 * D], F32, kind="ExternalInput").ap()
    br = nc.dram_tensor("br", [D], F32, kind="ExternalInput").ap()
    out = nc.dram_tensor("out", [BPC, D], F32, kind="ExternalOutput").ap()

    with tile.TileContext(nc) as tc:
        with (
            tc.tile_pool(name="consts", bufs=1) as consts,
            tc.tile_pool(name="dram", bufs=2, space="DRAM") as dpool,
            tc.tile_pool(name="res", bufs=1) as respool,
            tc.tile_pool(name="gstate", bufs=1) as gstate,
            tc.tile_pool(name="drep", bufs=1) as drep,
            tc.tile_pool(name="ytp", bufs=2) as ytp,
            tc.tile_pool(name="hTp", bufs=1) as hTp,
            tc.tile_pool(name="zpool", bufs=1) as zpool,
            tc.tile_pool(name="gath", bufs=2) as gath,
            tc.tile_pool(name="work", bufs=2) as work,
            tc.tile_pool(name="accp", bufs=2, space="PSUM") as accp,
        ):
            def ps(shape, name, dtype=F32):
                return accp.tile(shape, dtype, tag="acc", name=name)

            # ---------------- Phase 0: constants ----------------
            ident = consts.tile([128, 128], F32)
            make_identity(nc, ident[:])
            ident16 = consts.tile([128, 128], FP16)
            make_identity(nc, ident16[:])

            w1dup = consts.tile([128, D], FP16)
            nc.sync.dma_start(out=w1dup[0:D, :], in_=w1[:, :])
            nc.sync.dma_start(out=w1dup[D:2 * D, :], in_=w1[:, :])
            w2dup = consts.tile([128, D], FP16)
            nc.sync.dma_start(out=w2dup[0:D, :], in_=w2[:, :])
            nc.sync.dma_start(out=w2dup[D:2 * D, :], in_=w2[:, :])

            b1s = consts.tile([D, 1], F32)
            nc.sync.dma_start(out=b1s[:], in_=b1[:, None])
            b2s = consts.tile([D, 1], F32)
            nc.sync.dma_start(out=b2s[:], in_=b2[:, None])
            brs = consts.tile([1, D], F32)
            nc.sync.dma_start(out=brs[:], in_=br[None, :])

            ones16 = consts.tile([128, 2], FP16)
            nc.vector.memset(ones16[:], 1.0)
            halfN = consts.tile([128, 1], F32)
            nc.vector.memset(halfN[:], float(N) * 0.5)

            # Wr.T halves for the readout matmul (host pre-folds 1/N into
            # the mean half, so the raw column SUM feeds the matmul).
            wrs = consts.tile([D, 2 * D], F32)
            nc.sync.dma_start(out=wrs[:], in_=wr[:, :])
            wrmT = consts.tile([D, D], F32)
            wrxT = consts.tile([D, D], F32)
            for half, dst in ((0, wrmT), (1, wrxT)):
                tp = ps([D, D], f"wrt_ps{half}")
                nc.tensor.transpose(out=tp[:], in_=wrs[:, half * D:(half + 1) * D],
                                    identity=ident[:D, :D])
                nc.scalar.copy(out=dst[:], in_=tp[:])

            # ---------------- Per-graph pipeline ----------------
            pending_readout = None
            for g in range(BPC):
                # ---- index tile first (tiny; frees the gpsimd queue to run
                # the gathers immediately): idW[p, J] = fused_idx[J*128+p]
                t32 = work.tile([32, 128], I32, tag="id32", name=f"id32_{g}")
                src_ap = bass.AP(tensor=fidx.tensor, offset=g * N,
                                 ap=[[128, 32], [1, 128]])
                nc.sync.dma_start(out=t32[:], in_=src_ap)
                idW = work.tile([128, 32], I32, tag="idW", name=f"idW_{g}")
                for b_ in range(4):
                    nc.vector.transpose(out=idW[32 * b_:32 * (b_ + 1), :],
                                        in_=t32[:, 32 * b_:32 * (b_ + 1)])

                # fused-embedding gathers (fp16 rows, one per node tile)
                xg = gath.tile([128, NT * D], FP16, tag="xg", name=f"xg{g}")
                for J in range(NT):
                    nc.gpsimd.indirect_dma_start(
                        out=xg[:, J * D:(J + 1) * D], out_offset=None,
                        in_=xtab[:, :],
                        in_offset=bass.IndirectOffsetOnAxis(ap=idW[:, J:J + 1],
                                                            axis=0))

                # ---- resident fp8 A.T double-tiles (used by deg + BOTH
                # layers), split across BOTH HWDGE queues for bandwidth
                res2 = []
                for q in range(ND):
                    r = respool.tile([128, 2 * N], FP8, tag=f"res{q}",
                                     name=f"res{g}_{q}")
                    src = bass.AP(tensor=a_t.tensor,
                                  offset=(g * N + q * 256) * N,
                                  ap=[[N, 128], [128 * N, 2], [1, N]])
                    eng = nc.sync if q % 2 == 0 else nc.scalar
                    eng.dma_start(out=r[:], in_=src)
                    res2.append(r)

                def res_rhs(J, i0, i1):
                    return res2[J // 2][:, (J % 2) * N + i0:(J % 2) * N + i1]

                # ---- deg on the PE: deg_i = sum_j A'.T[j, i] via ones-matmul
                # over the resident tiles — 4-way column-group tiled so four
                # rhs streams run concurrently (4 cols/cycle instead of 1)
                dps = [ps([128, HALF], f"deg{g}_{h}") for h in range(2)]
                for J in range(NT):
                    for c in range(8):
                        i0 = c * 512
                        cg = c // 2            # col group 0..3
                        row = 32 * cg
                        nc.tensor.matmul(
                            out=dps[c // 4][row:row + 1,
                                            i0 % HALF:i0 % HALF + 512],
                            lhsT=ones16[:, 0:1],
                            rhs=res_rhs(J, i0, i0 + 512),
                            start=(J == 0), stop=(J == NT - 1),
                            tile_position=(0, row),
                            skip_group_check=True)
                # previous graph's readout slots in behind this graph's
                # (stream-gated) deg matmuls instead of blocking them
                if pending_readout is not None:
                    pending_readout()

                # drain the 4 lane-rows (deg for i-quarters at partitions
                # 0/32/64/96), then DMA them to a flat DRAM row; sqrt/recip
                # run on the [32, 128] reshape (128 elems/lane — cheap)
                QU = N // 4
                rows4 = gstate.tile([97, QU], F32, tag="rows4", name=f"rows4{g}")
                nc.scalar.copy(out=rows4[0:1, :], in_=dps[0][0:1, 0:QU])
                nc.vector.tensor_copy(out=rows4[32:33, :], in_=dps[0][32:33, QU:HALF])
                nc.scalar.copy(out=rows4[64:65, :], in_=dps[1][64:65, 0:QU])
                nc.vector.tensor_copy(out=rows4[96:97, :], in_=dps[1][96:97, QU:HALF])
                drow = dpool.tile([2, HALF], F32, tag="drow", name=f"drow{g}")
                for c4 in range(4):
                    nc.sync.dma_start(
                        out=drow[c4 // 2:c4 // 2 + 1,
                                 (c4 % 2) * QU:(c4 % 2 + 1) * QU],
                        in_=rows4[32 * c4:32 * c4 + 1, :])
                drow_ap = drow[:, :]
                dcol = work.tile([32, 128], F32, tag="dcol", name=f"dcol{g}")
                dcol_src = bass.AP(tensor=drow_ap.tensor, offset=drow_ap.offset,
                                   ap=[[128, 32], [1, 128]])
                nc.sync.dma_start(out=dcol[:], in_=dcol_src)
                nc.scalar.activation(out=dcol[:], in_=dcol[:], func=AF.Sqrt,
                                     bias=halfN[0:32, 0:1])
                dcol16 = work.tile([32, 128], FP16, tag="dcol16", name=f"dc16{g}")
                with nc.allow_low_precision(reason="dis ~0.02; fp16 rel err 5e-4"):
                    nc.vector.reciprocal(out=dcol16[:], in_=dcol[:])
                dis16 = work.tile([128, NT], FP16, tag="disnm", name=f"disnm{g}")
                for b_ in range(4):
                    nc.vector.transpose(out=dis16[32 * b_:32 * (b_ + 1), :],
                                        in_=dcol16[:, 32 * b_:32 * (b_ + 1)])
                # disrep (first needed at L1-post, off the z1 critical path)
                drow2 = dpool.tile([32, 128], FP16, tag="drow2", name=f"drow2{g}")
                nc.sync.dma_start(out=drow2[:, :], in_=dcol16[:])
                drow2_ap = drow2[:, :]
                bc = bass.AP(tensor=drow2_ap.tensor, offset=drow2_ap.offset,
                             ap=[[0, 128], [1, N]])
                disr = drep.tile([128, N], FP16, tag="disrep", name=f"disr{g}")
                nc.sync.dma_start(out=disr[:], in_=bc)

                # ---- z1 tiles: dis_j * xtab[fused_idx], fp16 (batched 8 node
                # tiles per DVE op via a stride-0 broadcast AP on dis)
                z1 = []
                for q8 in range(4):
                    zt8 = zpool.tile([128, 8 * D], FP16, tag=f"z1q{q8}",
                                     name=f"z1_{g}_{q8}")
                    dd = dis16[:, 8 * q8:8 * q8 + 8]
                    dbc = bass.AP(tensor=dd.tensor, offset=dd.offset,
                                  ap=[dd.ap[0], [dd.ap[1][0], 8], [0, D]])
                    xv = xg[:, q8 * 8 * D:(q8 + 1) * 8 * D]
                    xvv = bass.AP(tensor=xv.tensor, offset=xv.offset,
                                  ap=[xv.ap[0], [D, 8], [1, D]])
                    zvv = bass.AP(tensor=zt8[:].tensor, offset=zt8[:].offset,
                                  ap=[zt8[:].ap[0], [D, 8], [1, D]])
                    nc.vector.tensor_tensor(out=zvv, in0=xvv, in1=dbc,
                                            op=mybir.AluOpType.mult)
                    for k in range(8):
                        z1.append(zt8[:, k * D:(k + 1) * D])

                # ---- two GCN layers off the resident A.T
                zs = z1                      # lhsT provider: list of [128, D] fp16
                h2T = None
                sums = work.tile([D, 2], F32, tag="sums", name=f"sums{g}")
                mxs = work.tile([D, 2], F32, tag="mxs", name=f"mxs{g}")
                for ell in range(2):
                    wdup = w1dup if ell == 0 else w2dup
                    bcol = b1s if ell == 0 else b2s

                    # rank-1 centering correction FIRST (before the big
                    # accumulators claim both psum slots): cstack =
                    # 0.5 * sum_j z[j, :] (even tiles -> 0:64, odd -> 64:128)
                    cps = ps([128, 1], f"c{g}_{ell}")
                    for J in range(NT):
                        nc.tensor.matmul(
                            out=cps[64 * (J % 2):64 * (J % 2) + 64, 0:1],
                            lhsT=zs[J], rhs=ones16[:, 0:1],
                            start=(J < 2), stop=(J >= NT - 2),
                            tile_position=(0, 64 * (J % 2)),
                            skip_group_check=True)
                    cstack = work.tile([128, 1], F32, tag="cst", name=f"cst{g}_{ell}")
                    nc.scalar.activation(out=cstack[:], in_=cps[:], func=AF.Identity,
                                         scale=0.5)

                    # main accumulation: Y.T(partial) [2*64, N] over 32 j-tiles
                    # (p-outer so each z weight-load covers 8 matmuls; the two
                    # col-groups still overlap via the PE reorder window)
                    accs = [ps([128, HALF], f"acc{g}_{ell}_{h}") for h in range(2)]
                    for jp in range(NT // 2):
                        for p in range(2):
                            J = 2 * jp + p
                            zJ = zs[J]
                            for h in range(2):
                                for c in range(HALF // 512):
                                    i0 = h * HALF + c * 512
                                    nc.tensor.matmul(
                                        out=accs[h][64 * p:64 * (p + 1),
                                                    c * 512:(c + 1) * 512],
                                        lhsT=zJ,
                                        rhs=res_rhs(J, i0, i0 + 512),
                                        start=(jp == 0), stop=(jp == NT // 2 - 1),
                                        tile_position=(0, 64 * p),
                                        skip_group_check=True)

                    # post-chain, chunked by i-half so the two halves pipeline:
                    # drain -> +c -> *dis_i -> W-matmul -> relu+bias drain ->
                    # (L1 only) *dis_i -> node-major transposes -> z2 tiles
                    yt = ytp.tile([128, N], FP16, tag="yt", name=f"yt{g}_{ell}")
                    hT = hTp.tile([D, N], FP16, tag="hT", name=f"hT{g}_{ell}")
                    z2 = [None] * NT
                    for h in range(2):
                        hH = h * HALF
                        if h == 0:
                            nc.scalar.copy(out=yt[:, hH:hH + HALF], in_=accs[0][:])
                        else:
                            nc.vector.tensor_copy(out=yt[:, hH:hH + HALF],
                                                  in_=accs[1][:])
                        nc.vector.tensor_scalar_add(yt[:, hH:hH + HALF],
                                                    yt[:, hH:hH + HALF],
                                                    cstack[:, 0:1])
                        nc.vector.tensor_tensor(out=yt[:, hH:hH + HALF],
                                                in0=yt[:, hH:hH + HALF],
                                                in1=disr[:, hH:hH + HALF],
                                                op=mybir.AluOpType.mult)
                        wps = ps([64, HALF], f"w{g}_{ell}_{h}")
                        for c in range(HALF // 512):
                            nc.tensor.matmul(
                                out=wps[:, c * 512:(c + 1) * 512],
                                lhsT=wdup[:],
                                rhs=yt[:, hH + c * 512:hH + (c + 1) * 512],
                                start=True, stop=True)
                        if ell == 1:
                            # mean-pool rides the relu drain's accumulator;
                            # max-pool per half overlaps the other half's chain
                            nc.scalar.activation(out=hT[:, hH:hH + HALF],
                                                 in_=wps[:], func=AF.Relu,
                                                 bias=bcol[:, 0:1],
                                                 accum_out=sums[:, h:h + 1])
                            nc.vector.reduce_max(out=mxs[:, h:h + 1],
                                                 in_=hT[:, hH:hH + HALF],
                                                 axis=mybir.AxisListType.X)
                        else:
                            nc.scalar.activation(out=hT[:, hH:hH + HALF],
                                                 in_=wps[:], func=AF.Relu,
                                                 bias=bcol[:, 0:1])
                        if ell == 0:
                            nc.vector.tensor_tensor(out=hT[:, hH:hH + HALF],
                                                    in0=hT[:, hH:hH + HALF],
                                                    in1=disr[0:D, hH:hH + HALF],
                                                    op=mybir.AluOpType.mult)
                            for q in range(4 * h, 4 * h + 4):
                                tq = ps([128, 4 * D], f"tq{g}_{q}", dtype=FP16)
                                for k in range(4):
                                    J = 4 * q + k
                                    nc.tensor.transpose(
                                        out=tq[:, k * D:(k + 1) * D],
                                        in_=hT[:, J * 128:(J + 1) * 128],
                                        identity=ident16[:D, :D])
                                zq = zpool.tile([128, 4 * D], FP16, tag=f"z2q{q}",
                                                name=f"z2q{g}_{q}")
                                nc.scalar.copy(out=zq[:], in_=tq[:])
                                for k in range(4):
                                    z2[4 * q + k] = zq[:, k * D:(k + 1) * D]
                    if ell == 0:
                        zs = z2
                    else:
                        h2T = hT

                # ---- readout: deferred — emitted after the NEXT graph's
                # deg-matmuls so it never sits in the PE FIFO ahead of them
                # (it is off the critical path; total time = last graph's end)
                def mk_readout(g=g, sums=sums, mxs=mxs):
                    def emit():
                        sums2 = work.tile([D, 1], F32, tag="sums2", name=f"s2_{g}")
                        nc.vector.tensor_add(out=sums2[:], in0=sums[:, 0:1],
                                             in1=sums[:, 1:2])
                        mx = work.tile([D, 1], F32, tag="mx", name=f"mx{g}")
                        nc.vector.tensor_tensor(out=mx[:], in0=mxs[:, 0:1],
                                                in1=mxs[:, 1:2],
                                                op=mybir.AluOpType.max)
                        ops_ = ps([1, D], f"ops{g}")
                        nc.tensor.matmul(out=ops_[:], lhsT=sums2[:], rhs=wrmT[:],
                                         start=True, stop=False)
                        nc.tensor.matmul(out=ops_[:], lhsT=mx[:], rhs=wrxT[:],
                                         start=False, stop=True)
                        ob = work.tile([1, D], F32, tag="ob", name=f"ob{g}")
                        nc.vector.tensor_add(out=ob[:], in0=ops_[:], in1=brs[:])
                        nc.gpsimd.dma_start(out=out[g:g + 1, :], in_=ob[:])
                    return emit
                pending_readout = mk_readout()
            pending_readout()

    nc.compile()
    return nc


def _get_program():
    if "nc" not in _CACHE:
        _CACHE["nc"] = _build()
    return _CACHE["nc"]


def _shard_inputs(inputs):
    f32 = np.float32
    i32 = np.int32
    nt = np.ascontiguousarray(np.asarray(inputs["node_types"], dtype=i32))
    lb = np.ascontiguousarray(np.asarray(inputs["node_labels"], dtype=i32))
    adj = np.asarray(inputs["adj"], dtype=f32)

    wr = np.asarray(inputs["Wr"], dtype=f32).copy()
    wr[:, :D] *= 1.0 / N        # fold mean-pool 1/N into the readout weight

    # fused embedding table: row l*NTYPES+t = [type_emb[t] | label_emb[l]]
    te = np.asarray(inputs["type_emb"], dtype=f32)
    le = np.asarray(inputs["label_emb"], dtype=f32)
    xt = np.empty((VOCAB, NTYPES, D), dtype=np.float16)
    xt[:, :, :EMB] = te[None, :, :]
    xt[:, :, EMB:] = le[:, None, :]
    xtab = np.ascontiguousarray(xt.reshape(VOCAB * NTYPES, D))
    fidx = np.ascontiguousarray(lb * np.int32(NTYPES) + nt)

    rep = {
        "xtab": xtab,
        "W1h": np.ascontiguousarray(np.asarray(inputs["W1"], dtype=np.float16)),
        "W2h": np.ascontiguousarray(np.asarray(inputs["W2"], dtype=np.float16)),
        "b1": np.ascontiguousarray(np.asarray(inputs["b1"], dtype=f32)),
        "b2": np.ascontiguousarray(np.asarray(inputs["b2"], dtype=f32)),
        "Wr": np.ascontiguousarray(wr),
        "br": np.ascontiguousarray(np.asarray(inputs["br"], dtype=f32)),
    }
    in_maps = []
    for c in range(NCORES):
        s = slice(c * BPC, (c + 1) * BPC)
        ac = (adj[s] - np.float32(0.5)).astype(NP_FP8)
        at = np.ascontiguousarray(ac.transpose(0, 2, 1))
        in_maps.append({
            "a_t": at,
            "fused_idx": fidx[s],
            **rep,
        })
    return in_maps


def run_sharded(inputs, trace=False, **kw):
    """Returns (output [B, D] f32, BassKernelResults)."""
    nc = _get_program()
    in_maps = _shard_inputs(inputs)
    res = bass_utils.run_bass_kernel_spmd(nc, in_maps, core_ids=list(range(NCORES)),
                                          trace=trace, **kw)
    outp = np.concatenate([res.results[c]["out"] for c in range(NCORES)], axis=0)
    return outp.astype(np.float32), res


def kernel(**inputs) -> np.ndarray:
    outp, _ = run_sharded(inputs, trace=False)
    return outp
